# revision 5
# baseline (speedup 1.0000x reference)
"""Bass/Trainium2 kernel for nn_DecoderBlock (masked block-sparse linear +
BatchNorm(train) + Swish), sharded over C_OUT blocks across 8 NeuronCores.

Contract: kernel(**inputs) takes the FULL inputs from setup_inputs() and
returns the FULL [B, C_OUT, F_OUT] output.

Sharding: core k owns output channels [4k, 4k+4). With the reference's
block mask (o//4 == c//4) each core needs only input channels [4k, 4k+4),
so the useful slice of W (1/8 of it) is read from HBM exactly once across
the 8 cores, and every core holds the whole batch for its features =>
BatchNorm statistics are local (no collectives).

Math notes:
 - bias cancels exactly through BatchNorm's mean subtraction -> dropped.
 - MODE "bf16x3": matmul as 3 bf16 passes (W_hi@[x_hi|x_lo] with N=512 +
   W_lo@x_hi), ~2^-17 relative accuracy, faster than native fp32.
   MODE "f32r": single float32r pass (~2e-4 accuracy, 3x less PE time).
 - epilogue: y -> bn_stats/bn_aggr per tile right after its last matmul,
   batched Sqrt across tiles (one ACT table load), then back-to-back
   ScalarE Silu(a*y + c) with per-partition a = gamma*rstd,
   c = beta - mean*a.
"""

import os

import numpy as np
import ml_dtypes

B = 256
C_IN, F_IN = 32, 256
C_OUT, F_OUT = 32, 256
KERNEL_SIZE = 4
BN_EPS = 1e-5
N_CORES = 8
OC_PER_CORE = C_OUT // N_CORES  # 4 output channels per core
P = 128

MODE = os.environ.get("KERNEL_MODE", "bf16x3")  # "bf16x3" | "f32r"
TRACE = False  # set True (e.g. from test.py) to capture an NTFF profile
LAST_RESULT = {}  # exec_time_ns etc. from the most recent run

_program_cache = {}


def _build_program(kc, mode):
    """Build the SPMD Bass program for kc active input channels per core."""
    import concourse.bass as bass
    import concourse.tile as tile
    import concourse.mybir as mybir

    K = kc * F_IN  # contraction dim
    KT = K // P  # k-tiles of 128
    PT = (OC_PER_CORE * F_OUT) // P  # output-feature tiles of 128 (=8)
    NP = OC_PER_CORE * F_OUT  # per-core output features (=1024)
    f32 = mybir.dt.float32
    bf16 = mybir.dt.bfloat16
    f32r = mybir.dt.float32r
    AFT = mybir.ActivationFunctionType

    nc = bass.Bass()
    if mode == "bf16x3":
        # xc: [hi | lo] concatenated on the free dim -> one N=512 moving op
        xc_d = nc.declare_dram_parameter("xc", [KT, P, 2 * B], bf16, isOutput=False)
        wh_d = nc.declare_dram_parameter("wh", [KT, P, NP], bf16, isOutput=False)
        wl_d = nc.declare_dram_parameter("wl", [KT, P, NP], bf16, isOutput=False)
    else:
        xc_d = nc.declare_dram_parameter("xc", [KT, P, B], f32r, isOutput=False)
        wh_d = nc.declare_dram_parameter("wh", [KT, P, NP], f32r, isOutput=False)
    gs_d = nc.declare_dram_parameter("gs", [P, PT], f32, isOutput=False)
    bs_d = nc.declare_dram_parameter("bs", [P, PT], f32, isOutput=False)
    out_d = nc.declare_dram_parameter("out", [PT, P, B], f32, isOutput=True)

    with tile.TileContext(nc) as tc:
        with (
            tc.tile_pool(name="wpool", bufs=1) as wpool,
            tc.tile_pool(name="xpool", bufs=1) as xpool,
            tc.tile_pool(name="spool", bufs=1) as spool,
            tc.tile_pool(name="stat", bufs=1) as stat,
            tc.tile_pool(name="opool", bufs=1) as opool,
            tc.tile_pool(name="psum", bufs=1, space="PSUM") as psum,
        ):
            # Input DMAs in consumption order: k-tile 0 first, small stuff last.
            xc_t, wh_t, wl_t = [], [], []
            for kt in range(KT):
                if mode == "bf16x3":
                    t = xpool.tile([P, 2 * B], bf16, name=f"xc{kt}")
                else:
                    t = xpool.tile([P, B], f32r, name=f"xc{kt}")
                nc.sync.dma_start(out=t, in_=xc_d.ap()[kt])
                xc_t.append(t)
                t = wpool.tile(
                    [P, NP], bf16 if mode == "bf16x3" else f32r, name=f"wh{kt}"
                )
                nc.sync.dma_start(out=t, in_=wh_d.ap()[kt])
                wh_t.append(t)
                if mode == "bf16x3":
                    t = wpool.tile([P, NP], bf16, name=f"wl{kt}")
                    nc.sync.dma_start(out=t, in_=wl_d.ap()[kt])
                    wl_t.append(t)

            eps_t = spool.tile([P, 1], f32, name="eps")
            nc.vector.memset(eps_t, BN_EPS)
            gs_t = spool.tile([P, PT], f32, name="gs")
            nc.sync.dma_start(out=gs_t, in_=gs_d.ap())
            bs_t = spool.tile([P, PT], f32, name="bs")
            nc.sync.dma_start(out=bs_t, in_=bs_d.ap())

            if mode == "bf16x3":
                ps = [psum.tile([P, 2 * B], f32, name=f"ps{pt}") for pt in range(PT)]
            else:
                ps = [psum.tile([P, B], f32, name=f"ps{pt}") for pt in range(PT)]

            # Per-tile mean/var landing pad: mv_all[:, pt, 0]=mean, [:, pt, 1]=var
            mv_all = stat.tile([P, PT, 2], f32, name="mv_all")
            stats_t = [stat.tile([P, 6], f32, name=f"stats{pt}") for pt in range(PT)]
            y_sb = [stat.tile([P, B], f32, name=f"y{pt}") for pt in range(PT)]

            def tile_stats(pt):
                """Issued right after pt's last matmul: reduce psum -> y_sb,
                then batch-norm statistics for that tile. DVE may read only
                one PSUM operand per op, so stage the lo-half through SBUF."""
                if mode == "bf16x3":
                    nc.vector.tensor_copy(out=y_sb[pt], in_=ps[pt][:, B : 2 * B])
                    nc.vector.tensor_add(
                        out=y_sb[pt], in0=ps[pt][:, 0:B], in1=y_sb[pt]
                    )
                else:
                    nc.vector.tensor_copy(out=y_sb[pt], in_=ps[pt])
                nc.vector.bn_stats(out=stats_t[pt], in_=y_sb[pt])
                nc.vector.bn_aggr(out=mv_all[:, pt, :], in_=stats_t[pt])

            for kt in range(KT):
                first = kt == 0
                last = kt == KT - 1
                for pt in range(PT):
                    whs = wh_t[kt][:, pt * P : (pt + 1) * P]
                    nc.tensor.matmul(
                        ps[pt], lhsT=whs, rhs=xc_t[kt], start=first, stop=False
                    )
                    if mode == "bf16x3":
                        wls = wl_t[kt][:, pt * P : (pt + 1) * P]
                        nc.tensor.matmul(
                            ps[pt][:, 0:B],
                            lhsT=wls,
                            rhs=xc_t[kt][:, 0:B],
                            start=False,
                            stop=last,
                        )
                    if last:
                        tile_stats(pt)

            # Batched scale/shift: one Sqrt (one ACT table load) for all tiles.
            var_v = mv_all[:, :, 1]  # [P, PT] strided view
            mean_v = mv_all[:, :, 0]
            std_all = stat.tile([P, PT], f32, name="std_all")
            nc.scalar.activation(
                out=std_all, in_=var_v, func=AFT.Sqrt, bias=eps_t, scale=1.0
            )
            a_all = stat.tile([P, PT], f32, name="a_all")
            nc.vector.reciprocal(out=a_all, in_=std_all)
            nc.vector.tensor_mul(out=a_all, in0=a_all, in1=gs_t)
            c_all = stat.tile([P, PT], f32, name="c_all")
            nc.vector.tensor_mul(out=c_all, in0=mean_v, in1=a_all)
            nc.vector.tensor_sub(out=c_all, in0=bs_t, in1=c_all)

            for pt in range(PT):
                o_t = opool.tile([P, B], f32, name=f"o{pt}")
                nc.scalar.activation(
                    out=o_t,
                    in_=y_sb[pt],
                    func=AFT.Silu,
                    bias=c_all[:, pt : pt + 1],
                    scale=a_all[:, pt : pt + 1],
                )
                nc.sync.dma_start(out=out_d.ap()[pt], in_=o_t)

    _split_excess_waits(nc)
    return nc


def _split_excess_waits(nc, limit=1):
    """Walrus codegen rejects instructions carrying more than one sync wait;
    hoist excess waits onto same-engine NOPs inserted immediately before."""
    import concourse.mybir as mybir

    for fn in nc.m.functions:
        for blk in fn.blocks:
            new_insts = []
            for inst in blk.instructions:
                si = inst.sync_info
                waits = list(si.on_wait) if (si and si.on_wait) else []
                if len(waits) > limit:
                    extra = waits[:-limit]
                    inst.sync_info.on_wait = waits[-limit:]
                    while extra:
                        chunk, extra = extra[:limit], extra[limit:]
                        nop = mybir.InstNoOp(
                            name=nc.get_next_instruction_name(),
                            engine=inst.engine,
                            ins=[],
                            outs=[],
                            sync_info=mybir.SyncInfo(on_wait=chunk, on_update=[]),
                        )
                        new_insts.append(nop)
                new_insts.append(inst)
            blk.instructions[:] = new_insts


def _hi_lo(a):
    hi = a.astype(ml_dtypes.bfloat16)
    lo = (a - hi.astype(np.float32)).astype(ml_dtypes.bfloat16)
    return hi, lo


def kernel(x, W, bias, gamma, beta, mask):
    from concourse.bass_utils import run_bass_kernel_spmd

    x = np.asarray(x, dtype=np.float32)
    W = np.asarray(W, dtype=np.float32)
    gamma = np.asarray(gamma, dtype=np.float32)
    beta = np.asarray(beta, dtype=np.float32)
    mask_np = np.asarray(mask).astype(bool)

    groups = [
        list(range(OC_PER_CORE * k, OC_PER_CORE * (k + 1))) for k in range(N_CORES)
    ]
    active = [np.where(mask_np[g].any(axis=0))[0] for g in groups]
    kc = max(1, max(len(a) for a in active))

    key = (kc, MODE)
    if key not in _program_cache:
        _program_cache[key] = _build_program(kc, MODE)
    nc = _program_cache[key]

    K = kc * F_IN
    KT = K // P
    PT = (OC_PER_CORE * F_OUT) // P
    NP = OC_PER_CORE * F_OUT

    gamma2 = gamma.reshape(C_OUT, F_OUT)
    beta2 = beta.reshape(C_OUT, F_OUT)

    in_maps = []
    for k in range(N_CORES):
        g = groups[k]
        a = active[k]
        w_eff = np.zeros((OC_PER_CORE, kc, F_OUT, F_IN), dtype=np.float32)
        if len(a):
            w_eff[:, : len(a)] = W[g][:, a] * mask_np[g][:, a][:, :, None, None]
        # [k=(j,i), p=(o_local,f)]
        wT = np.ascontiguousarray(w_eff.transpose(1, 3, 0, 2).reshape(K, NP))
        xb = np.zeros((B, kc, F_IN), dtype=np.float32)
        if len(a):
            xb[:, : len(a)] = x[:, a, :]
        xT = np.ascontiguousarray(xb.transpose(1, 2, 0).reshape(K, B))

        g_core = gamma2[g].reshape(NP)  # ordered (o_local, f) = p
        b_core = beta2[g].reshape(NP)
        gs = np.ascontiguousarray(g_core.reshape(PT, P).T)  # [P, PT]
        bs = np.ascontiguousarray(b_core.reshape(PT, P).T)

        if MODE == "bf16x3":
            wh, wl = _hi_lo(wT)
            xh, xl = _hi_lo(xT)
            xc = np.concatenate(
                [xh.reshape(KT, P, B), xl.reshape(KT, P, B)], axis=2
            )
            in_maps.append(
                {
                    "xc": np.ascontiguousarray(xc),
                    "wh": np.ascontiguousarray(wh.reshape(KT, P, NP)),
                    "wl": np.ascontiguousarray(wl.reshape(KT, P, NP)),
                    "gs": gs,
                    "bs": bs,
                }
            )
        else:
            in_maps.append(
                {
                    "xc": np.ascontiguousarray(xT.reshape(KT, P, B)),
                    "wh": np.ascontiguousarray(wT.reshape(KT, P, NP)),
                    "gs": gs,
                    "bs": bs,
                }
            )

    res = run_bass_kernel_spmd(nc, in_maps, core_ids=list(range(N_CORES)), trace=TRACE)
    LAST_RESULT["exec_time_ns"] = res.exec_time_ns
    LAST_RESULT["mean_exec_time_ns"] = res.mean_exec_time_ns
    LAST_RESULT["trace"] = res.instructions_and_trace

    out = np.empty((B, C_OUT, F_OUT), dtype=np.float32)
    for k in range(N_CORES):
        y = res.results[k]["out"].reshape(NP, B)  # [p, b]
        out[:, groups[k], :] = y.T.reshape(B, OC_PER_CORE, F_OUT)
    return out
